# revision 39
# baseline (speedup 1.0000x reference)
"""Causal multi-head attention (b=2, s=2048, d=1024, h=16) on 8 TRN2 NeuronCores.

Sharding: DP=2 on batch x TP=4 on head groups (4 heads = 256 dims per core).
Host pre-transposes x and the weight slices so the device kernel is
transpose-free; the wo row-parallel partial sums + the bv/bo bias corrections
are applied on the host after gathering.

All matmul operands are bf16 (fp32 operands run in fp32_mode=HIGH, which
disables fast-weight-load and the background weight buffer -- every LDWEIGHTS
then serializes with the MM stream and row-group concurrency is lost; measured
335-423ns issue gaps at N=512 vs the 216ns bf16 floor).  PSUM accumulation
stays fp32; rel err vs the f32 oracle is ~3e-3 (budget 2e-2).

Device dataflow per core:
  xT [1024,2048] -> QT/KT [256,2048] (bias added on VectorE), V [2048,4x65]
  (65th column = ones, the stationary operand of the softmax denominator
  rows).  Per head pair and sq chunk: both heads' scoresT [sk,sq] land in one
  2-bank PSUM tile via concurrent row-group matmuls (head A in PE rows 0-63,
  B in 64-127), ONE merged exp per step on ScalarE (x1/8 folded into the
  activation scale) covering both heads, causal zeroing of the diag blocks on
  GpSimd post-exp, then the two AV+denominator matmuls accumulate per head.
  Softmax normalization runs as a deferred 3-stage pipeline (SBUF-local DMA
  respread -> 16-wide reciprocal -> GpSimd partition_broadcast -> muls), each
  stage emitted 1-2 attention steps after its inputs so no cross-engine wait
  ever parks at a FIFO queue head.

  The PE stream is issued in order; projection and wo matmuls are interleaved
  a few at a time between the scores and AV matmuls of every attention step
  to keep it dense through the ScalarE exp waits.
"""

import os

import numpy as np

D = 1024
S = 2048
B = 2
H = 16
DK = 64
TP = 4
DP = 2
EC = 256  # head dims per core
HPC = 4  # heads per core
NCORES = 8

TRACE = os.environ.get("KERNEL_TRACE", "0") == "1"
LAST_EXEC_NS = None

_compiled = {}


def _build_nc():
    import concourse.mybir as mybir
    from concourse import bacc, tile
    from concourse.bass import ts
    from itertools import chain

    f32 = mybir.dt.float32
    bf16 = mybir.dt.bfloat16
    AF = mybir.ActivationFunctionType

    nc = bacc.Bacc("TRN2", target_bir_lowering=False, debug=False)

    # Inputs are host-pre-swizzled to the exact SBUF layouts so every input
    # DMA is one fully-contiguous read (strided reads + many small DMAs made
    # the prologue DMA-latency-bound).
    xt_d = nc.dram_tensor(
        "xt", [128, S // 512, D // 128, 512], bf16, kind="ExternalInput"
    ).ap()
    wqt_d = nc.dram_tensor("wqt", [128, D // 128, EC], bf16, kind="ExternalInput").ap()
    wkt_d = nc.dram_tensor("wkt", [128, D // 128, EC], bf16, kind="ExternalInput").ap()
    wvt_d = nc.dram_tensor("wvt", [128, D // 128, EC], bf16, kind="ExternalInput").ap()
    wot_d = nc.dram_tensor("wot", [128, EC // 128, D], bf16, kind="ExternalInput").ap()
    bq_d = nc.dram_tensor("bq", [EC], f32, kind="ExternalInput").ap()
    bk_d = nc.dram_tensor("bk", [EC], f32, kind="ExternalInput").ap()
    out_d = nc.dram_tensor("out", [S, D], bf16, kind="ExternalOutput").ap()

    KT = D // 128  # 8 contraction tiles
    NC_SQ = S // 512  # 4 sq chunks

    with tile.TileContext(nc) as tc:
        with (
            tc.tile_pool(name="persist", bufs=1) as persist,
            tc.tile_pool(name="work", bufs=1) as work,
            tc.tile_pool(name="psum", bufs=1, space="PSUM") as psum,
            tc.tile_pool(name="dram", bufs=2, space="DRAM") as dram,
        ):
            # ---- persistent SBUF tensors ----
            xt_sb = persist.tile([128, KT, S], bf16)  # x^T, d on partitions
            wqt_sb = persist.tile([128, KT, EC], bf16)
            wkt_sb = persist.tile([128, KT, EC], bf16)
            wvt_sb = persist.tile([128, KT, EC], bf16)
            wot_sb = persist.tile([128, 2, D], bf16)
            bq_sb = persist.tile([128, 2], f32)
            bk_sb = persist.tile([128, 2], f32)
            qt_sb = persist.tile([128, 2, S], bf16)  # head pairs stacked
            kt_sb = persist.tile([128, 2, S], bf16)
            v_sb = persist.tile([128, S // 128, HPC * (DK + 1)], bf16)
            avt_sb = persist.tile([128, 2, S], bf16)

            # ---- input DMAs: one large contiguous transfer per tensor
            # (per-DMA fixed cost is ~0.6-2us and HWDGE drains FIFO, so many
            # small DMAs serialize), ordered so chunk-0 work starts ASAP ----
            nc.sync.dma_start(out=wqt_sb, in_=wqt_d)
            nc.sync.dma_start(out=xt_sb[:, :, ts(0, 512)], in_=xt_d[:, 0])
            nc.sync.dma_start(out=wkt_sb, in_=wkt_d)
            nc.sync.dma_start(out=wvt_sb, in_=wvt_d)
            nc.sync.dma_start(out=bq_sb, in_=bq_d.rearrange("(t p) -> p t", p=128))
            nc.sync.dma_start(out=bk_sb, in_=bk_d.rearrange("(t p) -> p t", p=128))
            for c in range(1, NC_SQ):
                nc.sync.dma_start(out=xt_sb[:, :, ts(c, 512)], in_=xt_d[:, c])
            nc.sync.dma_start(out=wot_sb, in_=wot_d)

            # ones column per head in V (stationary operand of the denom
            # matmul rows).
            v4 = v_sb.rearrange("p t (h e) -> p t h e", e=DK + 1)
            nc.vector.memset(v4[:, :, :, DK], 1.0)

            # Startup warm-up: DMA packets only start flowing ~9us into the
            # kernel, and the first ~3.4us of matmuls would run at the cold
            # 1.2GHz HAM clock.  Dependency-free dummy matmuls on memset
            # tiles warm the PE clock during the DMA dead zone and bridge
            # until the critical inputs (wq+wk+x chunk 0) have landed.
            dumw = persist.tile([128, 128], bf16)
            dumx = persist.tile([128, 512], bf16)
            nc.vector.memset(dumw, 0.0)
            nc.vector.memset(dumx, 0.0)
            pdum0 = psum.tile([128, 512], f32, tag="score", bufs=2)
            for k in range(14):
                nc.tensor.matmul(
                    pdum0, lhsT=dumw, rhs=dumx, start=(k == 0), stop=(k == 13)
                )

            def qtkt_gen(c):
                """QT/KT projections for chunk c; yields once per matmul."""
                for dst_sb, w_sb, b_sb in (
                    (qt_sb, wqt_sb, bq_sb),
                    (kt_sb, wkt_sb, bk_sb),
                ):
                    for d2 in range(2):
                        ps = psum.tile([128, 512], f32, tag="proj", bufs=2)
                        for k in range(KT):
                            nc.tensor.matmul(
                                ps,
                                lhsT=w_sb[:, k, ts(d2, 128)],
                                rhs=xt_sb[:, k, ts(c, 512)],
                                start=(k == 0),
                                stop=(k == KT - 1),
                            )
                            if k == KT - 1:
                                nc.vector.tensor_scalar_add(
                                    out=dst_sb[:, d2, ts(c, 512)],
                                    in0=ps,
                                    scalar1=b_sb[:, d2 : d2 + 1],
                                )
                            yield

            def v_gen(tiles):
                """V projection for the given s-tiles; yields once per matmul."""
                for t in tiles:
                    ps = psum.tile([128, EC], f32, tag="proj", bufs=2)
                    for k in range(KT):
                        nc.tensor.matmul(
                            ps,
                            lhsT=xt_sb[:, k, ts(t, 128)],
                            rhs=wvt_sb[:, k, :],
                            start=(k == 0),
                            stop=(k == KT - 1),
                        )
                        if k == KT - 1:
                            nc.vector.tensor_copy(
                                out=v4[:, t, :, 0:DK],
                                in_=ps.rearrange("p (h e) -> p h e", e=DK),
                            )
                        yield

            def wo_gen(c):
                for t in range(4 * c, 4 * c + 4):
                    osb = work.tile([128, D], bf16, tag="osb", bufs=2)
                    for n in range(2):
                        po = psum.tile([128, 512], f32, tag="proj", bufs=2)
                        for p2 in range(2):
                            nc.tensor.matmul(
                                po,
                                lhsT=avt_sb[:, p2, ts(t, 128)],
                                rhs=wot_sb[:, p2, ts(n, 512)],
                                start=(p2 == 0),
                                stop=(p2 == 1),
                            )
                            if p2 == 1:
                                nc.vector.tensor_copy(
                                    out=osb[:, ts(n, 512)], in_=po
                                )
                                nc.sync.dma_start(
                                    out=out_d[ts(t, 128), ts(n, 512)],
                                    in_=osb[:, ts(n, 512)],
                                )
                            yield

            def drain(gen, n=None):
                took = 0
                for _ in gen:
                    took += 1
                    if n is not None and took >= n:
                        break
                return took

            # Deferred-normalize stages.  Every op in the chain (reciprocal,
            # broadcast, final muls) waits on a DMA or cross-engine result;
            # emitted inline they park at their engine's FIFO queue head and
            # block everything behind them (measured 6.7us DVE-head stalls
            # that starved the PE).  Instead each stage is emitted 1-2
            # attention steps AFTER its inputs were produced, so by the time
            # it reaches the queue head its inputs are long done.
            norm_q = []

            def attention_chunk(c, filler, quota):
                for pr in range(2):
                    pa = psum.tile([65, 512], f32, tag="av", bufs=2)
                    pb = psum.tile([65, 512], f32, tag="av", bufs=2)
                    n_sk = 4 * c + 4
                    for i in range(n_sk):
                        off = max(0, 128 * i - 512 * c)
                        w = 512 - off
                        sq_lo = 512 * c + off
                        # both heads' scoresT into one 2-bank psum tile:
                        # A via PE rows 0-63, B via rows 64-127 (concurrent)
                        pscore = psum.tile([128, 2, 512], f32, tag="score", bufs=2)
                        for hh, (p_lo, p_hi) in enumerate(((0, 64), (64, 128))):
                            nc.tensor.matmul(
                                pscore[:, hh, 0:w],
                                lhsT=kt_sb[p_lo:p_hi, pr, ts(i, 128)],
                                rhs=qt_sb[p_lo:p_hi, pr, sq_lo : sq_lo + w],
                                start=True,
                                stop=True,
                            )
                        et = work.tile([128, 2, 512], bf16, tag="exp", bufs=4)
                        nc.scalar.activation(
                            out=et[:, :, 0:w],
                            in_=pscore[:, :, 0:w],
                            func=AF.Exp,
                            scale=0.125,
                        )
                        if i >= 4 * c:
                            # causal: zero the lower triangle of the diag
                            # block post-exp on the otherwise-idle GpSimd
                            for hh in range(2):
                                dv = et[:, hh, 0:128]
                                nc.gpsimd.affine_select(
                                    out=dv,
                                    in_=dv,
                                    compare_op=mybir.AluOpType.is_ge,
                                    fill=0.0,
                                    base=0,
                                    pattern=[[1, 128]],
                                    channel_multiplier=-1,
                                )
                        # keep the in-order PE stream fed while the exp runs
                        q = quota + (2 if (i == 0 and not (c == 0 and pr == 0)) else 0)
                        drain(filler, q)
                        if norm_q:
                            st = norm_q.pop(0)
                            if st is not None:
                                st()
                        for hh, p_av in enumerate((pa, pb)):
                            h = 2 * pr + hh
                            nc.tensor.matmul(
                                p_av[:, off : off + w],
                                lhsT=v_sb[:, i, h * 65 : h * 65 + 65],
                                rhs=et[:, hh, 0:w],
                                start=(i == 0),
                                stop=(i == n_sk - 1),
                                skip_group_check=True,
                            )
                    # Stage 1, inline: copy the finished accumulators to SBUF
                    # (frees the two av PSUM banks after ~1.4us of DVE so the
                    # next pair's AV matmuls never wait on the chain), then
                    # SBUF->SBUF DMAs respread the denominator rows over 64
                    # partitions x 8 so the DVE iterative-divide reciprocal
                    # can run on a 16-wide free dim (290ns vs 3.3us at 512).
                    sa = work.tile([65, 512], f32, tag="avsb", bufs=4)
                    sb2 = work.tile([65, 512], f32, tag="avsb", bufs=4)
                    nc.vector.tensor_copy(out=sa, in_=pa)
                    nc.vector.tensor_copy(out=sb2, in_=pb)
                    den2 = work.tile([64, 16], f32, tag="den2", bufs=2)
                    nc.sync.dma_start(out=den2[:, 0:8], in_=sa[64:65, :])
                    nc.sync.dma_start(out=den2[:, 8:16], in_=sb2[64:65, :])
                    rec_row = work.tile([1, 1024], f32, tag="rec_row", bufs=2)
                    recbs = [
                        work.tile([64, 512], f32, tag="recb", bufs=2, name="recb_a"),
                        work.tile([64, 512], f32, tag="recb", bufs=2, name="recb_b"),
                    ]

                    def stage2a(den2=den2, rec_row=rec_row):
                        # reciprocal on the 16-wide respread, regather to one
                        # partition row for the broadcast
                        rec2 = work.tile([64, 16], f32, tag="rec2", bufs=2)
                        nc.vector.reciprocal(out=rec2, in_=den2)
                        for hh in range(2):
                            nc.sync.dma_start(
                                out=rec_row[0:1, 512 * hh : 512 * hh + 512],
                                in_=rec2[:, 8 * hh : 8 * hh + 8],
                            )

                    def stage2b(rec_row=rec_row, recbs=recbs):
                        # GpSimd partition_broadcast fans 1/denom out across
                        # 64 partitions (SBUF APs can't have a 0 partition
                        # step, so a plain DVE op can't do this).
                        for hh in range(2):
                            nc.gpsimd.partition_broadcast(
                                out_ap=recbs[hh],
                                in_ap=rec_row[0:1, 512 * hh : 512 * hh + 512],
                            )

                    def stage3(recbs=recbs, sa=sa, sb2=sb2, pr=pr, c=c):
                        for hh, s_av in enumerate((sa, sb2)):
                            nc.vector.tensor_mul(
                                out=avt_sb[64 * hh : 64 * hh + 64, pr, ts(c, 512)],
                                in0=s_av[0:64, :],
                                in1=recbs[hh],
                            )

                    if c == 0:
                        # chunk-0 prs are only 4 steps (~5us) -- not enough
                        # ticks for the DMA chain latencies; space further out
                        norm_q.extend(
                            [None, stage2a, None, None, stage2b, None, None, stage3]
                        )
                    else:
                        norm_q.extend(
                            [None, stage2a, None, stage2b, None, stage3]
                        )

            # Filler plan (PE work interleaved into the attention steps, paced
            # so no chunk starves):
            #   prologue: QT/KT+V for chunk 0 (dense)
            #   chunk 0: QT/KT(1) + V tiles 4,5        (48 mms, 8 steps)
            #   chunk 1: V 6,7 + QT/KT(2) + V 8,9      (64 mms, 16 steps)
            #   chunk 2: V 10,11 + QT/KT(3) + V 12,13  (64 mms, 24 steps)
            #   chunk 3: V 14,15 + wo(0..2)            (64 mms, 32 steps)
            #   tail: wo(3)
            drain(qtkt_gen(0))
            drain(v_gen(range(0, 4)))
            plans = [
                (chain(qtkt_gen(1), v_gen([4, 5])), 5),
                (chain(v_gen([6, 7]), qtkt_gen(2), v_gen([8, 9])), 3),
                (chain(v_gen([10, 11]), qtkt_gen(3), v_gen([12, 13])), 2),
                (chain(v_gen([14, 15]), wo_gen(0), wo_gen(1), wo_gen(2)), 2),
            ]
            for c in range(NC_SQ):
                filler, quota = plans[c]
                attention_chunk(c, filler, quota)
                drain(filler)  # anything attention didn't absorb
            while norm_q:  # tail: flush the last pair's normalize stages
                st = norm_q.pop(0)
                if st is not None:
                    st()
            # dependency-free dummy matmuls keep the PE busy through the
            # final normalize chain so HAM doesn't re-throttle the clock to
            # 1.2GHz right before the last wo tile's matmuls
            pdum = psum.tile([128, 512], f32, tag="score", bufs=2)
            for k in range(36):
                nc.tensor.matmul(
                    pdum,
                    lhsT=wot_sb[:, 0, ts(0, 128)],
                    rhs=xt_sb[:, 0, ts(0, 512)],
                    start=(k == 0),
                    stop=(k == 35),
                )
            drain(wo_gen(NC_SQ - 1))

    nc.compile()
    return nc


def _get_nc():
    if "nc" not in _compiled:
        _compiled["nc"] = _build_nc()
    return _compiled["nc"]


def _swz(mat_t, kt):
    """[kt*128, m] -> [128, kt, m] contiguous (SBUF layout, d on partitions)."""
    import ml_dtypes

    m = mat_t.shape[1]
    return np.ascontiguousarray(
        mat_t.reshape(kt, 128, m).transpose(1, 0, 2)
    ).astype(ml_dtypes.bfloat16)


def make_in_maps(x, wq, bq, wk, bk, wv, bv, wo, bo):
    """Host-side sharding: core c handles batch c//TP, heads 4*(c%TP)..+4.
    Matrices are pre-swizzled to the exact SBUF layouts so device input DMAs
    are single fully-contiguous reads."""
    import ml_dtypes

    bf16 = ml_dtypes.bfloat16
    in_maps = []
    for c in range(NCORES):
        b = c // TP
        hs = (c % TP) * EC
        he = hs + EC
        xt = x[b].T  # [D, S]
        # [128, S//512 chunks, D//128 ktiles, 512]
        xt_swz = np.ascontiguousarray(
            xt.reshape(D // 128, 128, S // 512, 512).transpose(1, 2, 0, 3)
        ).astype(bf16)
        in_maps.append(
            {
                "xt": xt_swz,
                "wqt": _swz(np.ascontiguousarray(wq[hs:he, :].T), D // 128),
                "wkt": _swz(np.ascontiguousarray(wk[hs:he, :].T), D // 128),
                "wvt": _swz(np.ascontiguousarray(wv[hs:he, :].T), D // 128),
                "wot": _swz(np.ascontiguousarray(wo[:, hs:he].T), EC // 128),
                "bq": np.ascontiguousarray(bq[hs:he]),
                "bk": np.ascontiguousarray(bk[hs:he]),
            }
        )
    return in_maps


def combine_outputs(partials, wo, bv, bo):
    """Host-side unsharding: sum TP partials per batch + bias correction."""
    corr = (wo.astype(np.float32) @ bv.astype(np.float32)) + bo.astype(np.float32)
    out = np.zeros((B, S, D), dtype=np.float32)
    for b in range(B):
        acc = np.zeros((S, D), dtype=np.float32)
        for g in range(TP):
            acc += np.asarray(partials[b * TP + g], dtype=np.float32)
        out[b] = acc + corr[None, :]
    return out


def kernel(x, wq, bq, wk, bk, wv, bv, wo, bo):
    global LAST_EXEC_NS
    from concourse.bass_utils import run_bass_kernel_spmd

    x = np.asarray(x, dtype=np.float32)
    wq = np.asarray(wq, dtype=np.float32)
    bq = np.asarray(bq, dtype=np.float32)
    wk = np.asarray(wk, dtype=np.float32)
    bk = np.asarray(bk, dtype=np.float32)
    wv = np.asarray(wv, dtype=np.float32)
    bv = np.asarray(bv, dtype=np.float32)
    wo = np.asarray(wo, dtype=np.float32)
    bo = np.asarray(bo, dtype=np.float32)

    nc = _get_nc()
    in_maps = make_in_maps(x, wq, bq, wk, bk, wv, bv, wo, bo)
    res = run_bass_kernel_spmd(
        nc, in_maps, core_ids=list(range(NCORES)), trace=TRACE
    )
    LAST_EXEC_NS = res.exec_time_ns
    _compiled["last_res"] = res
    partials = [res.results[c]["out"] for c in range(NCORES)]
    return combine_outputs(partials, wo, bv, bo)


# revision 40
# speedup vs baseline: 1.1808x; 1.1808x over previous
"""Causal multi-head attention (b=2, s=2048, d=1024, h=16) on 8 TRN2 NeuronCores.

Sharding: DP=2 on batch x TP=4 on head groups (4 heads = 256 dims per core).
Host pre-transposes x and the weight slices so the device kernel is
transpose-free; the wo row-parallel partial sums + the bv/bo bias corrections
are applied on the host after gathering.

All matmul operands are bf16 (fp32 operands run in fp32_mode=HIGH, which
disables fast-weight-load and the background weight buffer -- every LDWEIGHTS
then serializes with the MM stream and row-group concurrency is lost; measured
335-423ns issue gaps at N=512 vs the 216ns bf16 floor).  PSUM accumulation
stays fp32; rel err vs the f32 oracle is ~3e-3 (budget 2e-2).

Device dataflow per core:
  xT [1024,2048] -> QT/KT [256,2048] (bias added on VectorE), V [2048,4x65]
  (65th column = ones, the stationary operand of the softmax denominator
  rows).  Per head pair and sq chunk: both heads' scoresT [sk,sq] land in one
  2-bank PSUM tile via concurrent row-group matmuls (head A in PE rows 0-63,
  B in 64-127), ONE merged exp per step on ScalarE (x1/8 folded into the
  activation scale) covering both heads, causal zeroing of the diag blocks on
  GpSimd post-exp, then the two AV+denominator matmuls accumulate per head.
  Softmax normalization runs as a deferred 3-stage pipeline (SBUF-local DMA
  respread -> 16-wide reciprocal -> GpSimd partition_broadcast -> muls), each
  stage emitted 1-2 attention steps after its inputs so no cross-engine wait
  ever parks at a FIFO queue head.

  The PE stream is issued in order; projection and wo matmuls are interleaved
  a few at a time between the scores and AV matmuls of every attention step
  to keep it dense through the ScalarE exp waits.
"""

import os

import numpy as np

D = 1024
S = 2048
B = 2
H = 16
DK = 64
TP = 4
DP = 2
EC = 256  # head dims per core
HPC = 4  # heads per core
NCORES = 8

TRACE = os.environ.get("KERNEL_TRACE", "0") == "1"
LAST_EXEC_NS = None

_compiled = {}


def _build_nc():
    import concourse.mybir as mybir
    from concourse import bacc, tile
    from concourse.bass import ts
    from itertools import chain

    f32 = mybir.dt.float32
    bf16 = mybir.dt.bfloat16
    AF = mybir.ActivationFunctionType

    nc = bacc.Bacc("TRN2", target_bir_lowering=False, debug=False)

    # Inputs are host-pre-swizzled to the exact SBUF layouts so every input
    # DMA is one fully-contiguous read (strided reads + many small DMAs made
    # the prologue DMA-latency-bound).
    xt_d = nc.dram_tensor(
        "xt", [128, S // 512, D // 128, 512], bf16, kind="ExternalInput"
    ).ap()
    wqt_d = nc.dram_tensor("wqt", [128, D // 128, EC], bf16, kind="ExternalInput").ap()
    wkt_d = nc.dram_tensor("wkt", [128, D // 128, EC], bf16, kind="ExternalInput").ap()
    wvt_d = nc.dram_tensor("wvt", [128, D // 128, EC], bf16, kind="ExternalInput").ap()
    wot_d = nc.dram_tensor("wot", [128, EC // 128, D], bf16, kind="ExternalInput").ap()
    bq_d = nc.dram_tensor("bq", [EC], f32, kind="ExternalInput").ap()
    bk_d = nc.dram_tensor("bk", [EC], f32, kind="ExternalInput").ap()
    out_d = nc.dram_tensor("out", [S, D], bf16, kind="ExternalOutput").ap()

    KT = D // 128  # 8 contraction tiles
    NC_SQ = S // 512  # 4 sq chunks

    with tile.TileContext(nc) as tc:
        with (
            tc.tile_pool(name="persist", bufs=1) as persist,
            tc.tile_pool(name="work", bufs=1) as work,
            tc.tile_pool(name="psum", bufs=1, space="PSUM") as psum,
            tc.tile_pool(name="dram", bufs=2, space="DRAM") as dram,
        ):
            # ---- persistent SBUF tensors ----
            xt_sb = persist.tile([128, KT, S], bf16)  # x^T, d on partitions
            wqt_sb = persist.tile([128, KT, EC], bf16)
            wkt_sb = persist.tile([128, KT, EC], bf16)
            wvt_sb = persist.tile([128, KT, EC], bf16)
            wot_sb = persist.tile([128, 2, D], bf16)
            bq_sb = persist.tile([128, 2], f32)
            bk_sb = persist.tile([128, 2], f32)
            qt_sb = persist.tile([128, 2, S], bf16)  # head pairs stacked
            kt_sb = persist.tile([128, 2, S], bf16)
            v_sb = persist.tile([128, S // 128, HPC * (DK + 1)], bf16)
            avt_sb = persist.tile([128, 2, S], bf16)

            # ---- input DMAs: one large contiguous transfer per tensor
            # (per-DMA fixed cost is ~0.6-2us and HWDGE drains FIFO, so many
            # small DMAs serialize), ordered so chunk-0 work starts ASAP ----
            nc.sync.dma_start(out=wqt_sb, in_=wqt_d)
            nc.sync.dma_start(out=xt_sb[:, :, ts(0, 512)], in_=xt_d[:, 0])
            nc.sync.dma_start(out=wkt_sb, in_=wkt_d)
            nc.sync.dma_start(out=wvt_sb, in_=wvt_d)
            nc.sync.dma_start(out=bq_sb, in_=bq_d.rearrange("(t p) -> p t", p=128))
            nc.sync.dma_start(out=bk_sb, in_=bk_d.rearrange("(t p) -> p t", p=128))
            for c in range(1, NC_SQ):
                nc.sync.dma_start(out=xt_sb[:, :, ts(c, 512)], in_=xt_d[:, c])
            nc.sync.dma_start(out=wot_sb, in_=wot_d)

            # ones column per head in V (stationary operand of the denom
            # matmul rows).
            v4 = v_sb.rearrange("p t (h e) -> p t h e", e=DK + 1)
            nc.vector.memset(v4[:, :, :, DK], 1.0)



            def qtkt_gen(c):
                """QT/KT projections for chunk c; yields once per matmul."""
                for dst_sb, w_sb, b_sb in (
                    (qt_sb, wqt_sb, bq_sb),
                    (kt_sb, wkt_sb, bk_sb),
                ):
                    for d2 in range(2):
                        ps = psum.tile([128, 512], f32, tag="proj", bufs=2)
                        for k in range(KT):
                            nc.tensor.matmul(
                                ps,
                                lhsT=w_sb[:, k, ts(d2, 128)],
                                rhs=xt_sb[:, k, ts(c, 512)],
                                start=(k == 0),
                                stop=(k == KT - 1),
                            )
                            if k == KT - 1:
                                nc.vector.tensor_scalar_add(
                                    out=dst_sb[:, d2, ts(c, 512)],
                                    in0=ps,
                                    scalar1=b_sb[:, d2 : d2 + 1],
                                )
                            yield

            def v_gen(tiles):
                """V projection for the given s-tiles; yields once per matmul."""
                for t in tiles:
                    ps = psum.tile([128, EC], f32, tag="proj", bufs=2)
                    for k in range(KT):
                        nc.tensor.matmul(
                            ps,
                            lhsT=xt_sb[:, k, ts(t, 128)],
                            rhs=wvt_sb[:, k, :],
                            start=(k == 0),
                            stop=(k == KT - 1),
                        )
                        if k == KT - 1:
                            nc.vector.tensor_copy(
                                out=v4[:, t, :, 0:DK],
                                in_=ps.rearrange("p (h e) -> p h e", e=DK),
                            )
                        yield

            def wo_gen(c):
                for t in range(4 * c, 4 * c + 4):
                    osb = work.tile([128, D], bf16, tag="osb", bufs=2)
                    for n in range(2):
                        po = psum.tile([128, 512], f32, tag="proj", bufs=2)
                        for p2 in range(2):
                            nc.tensor.matmul(
                                po,
                                lhsT=avt_sb[:, p2, ts(t, 128)],
                                rhs=wot_sb[:, p2, ts(n, 512)],
                                start=(p2 == 0),
                                stop=(p2 == 1),
                            )
                            if p2 == 1:
                                nc.vector.tensor_copy(
                                    out=osb[:, ts(n, 512)], in_=po
                                )
                                nc.sync.dma_start(
                                    out=out_d[ts(t, 128), ts(n, 512)],
                                    in_=osb[:, ts(n, 512)],
                                )
                            yield

            def drain(gen, n=None):
                took = 0
                for _ in gen:
                    took += 1
                    if n is not None and took >= n:
                        break
                return took

            # Deferred-normalize stages.  Every op in the chain (reciprocal,
            # broadcast, final muls) waits on a DMA or cross-engine result;
            # emitted inline they park at their engine's FIFO queue head and
            # block everything behind them (measured 6.7us DVE-head stalls
            # that starved the PE).  Instead each stage is emitted 1-2
            # attention steps AFTER its inputs were produced, so by the time
            # it reaches the queue head its inputs are long done.
            norm_q = []

            def attention_chunk(c, filler, quota):
                for pr in range(2):
                    pa = psum.tile([65, 512], f32, tag="av", bufs=2)
                    pb = psum.tile([65, 512], f32, tag="av", bufs=2)
                    n_sk = 4 * c + 4
                    for i in range(n_sk):
                        off = max(0, 128 * i - 512 * c)
                        w = 512 - off
                        sq_lo = 512 * c + off
                        # both heads' scoresT into one 2-bank psum tile:
                        # A via PE rows 0-63, B via rows 64-127 (concurrent)
                        pscore = psum.tile([128, 2, 512], f32, tag="score", bufs=2)
                        for hh, (p_lo, p_hi) in enumerate(((0, 64), (64, 128))):
                            nc.tensor.matmul(
                                pscore[:, hh, 0:w],
                                lhsT=kt_sb[p_lo:p_hi, pr, ts(i, 128)],
                                rhs=qt_sb[p_lo:p_hi, pr, sq_lo : sq_lo + w],
                                start=True,
                                stop=True,
                            )
                        et = work.tile([128, 2, 512], bf16, tag="exp", bufs=4)
                        nc.scalar.activation(
                            out=et[:, :, 0:w],
                            in_=pscore[:, :, 0:w],
                            func=AF.Exp,
                            scale=0.125,
                        )
                        if i >= 4 * c:
                            # causal: zero the lower triangle of the diag
                            # block post-exp on the otherwise-idle GpSimd
                            for hh in range(2):
                                dv = et[:, hh, 0:128]
                                nc.gpsimd.affine_select(
                                    out=dv,
                                    in_=dv,
                                    compare_op=mybir.AluOpType.is_ge,
                                    fill=0.0,
                                    base=0,
                                    pattern=[[1, 128]],
                                    channel_multiplier=-1,
                                )
                        # keep the in-order PE stream fed while the exp runs
                        q = quota + (2 if (i == 0 and not (c == 0 and pr == 0)) else 0)
                        drain(filler, q)
                        if norm_q:
                            st = norm_q.pop(0)
                            if st is not None:
                                st()
                        for hh, p_av in enumerate((pa, pb)):
                            h = 2 * pr + hh
                            nc.tensor.matmul(
                                p_av[:, off : off + w],
                                lhsT=v_sb[:, i, h * 65 : h * 65 + 65],
                                rhs=et[:, hh, 0:w],
                                start=(i == 0),
                                stop=(i == n_sk - 1),
                                skip_group_check=True,
                            )
                    # Stage 1, inline: copy the finished accumulators to SBUF
                    # (frees the two av PSUM banks after ~1.4us of DVE so the
                    # next pair's AV matmuls never wait on the chain), then
                    # SBUF->SBUF DMAs respread the denominator rows over 64
                    # partitions x 8 so the DVE iterative-divide reciprocal
                    # can run on a 16-wide free dim (290ns vs 3.3us at 512).
                    sa = work.tile([65, 512], f32, tag="avsb", bufs=4)
                    sb2 = work.tile([65, 512], f32, tag="avsb", bufs=4)
                    nc.vector.tensor_copy(out=sa, in_=pa)
                    nc.vector.tensor_copy(out=sb2, in_=pb)
                    den2 = work.tile([64, 16], f32, tag="den2", bufs=2)
                    nc.sync.dma_start(out=den2[:, 0:8], in_=sa[64:65, :])
                    nc.sync.dma_start(out=den2[:, 8:16], in_=sb2[64:65, :])
                    rec_row = work.tile([1, 1024], f32, tag="rec_row", bufs=2)
                    recbs = [
                        work.tile([64, 512], f32, tag="recb", bufs=2, name="recb_a"),
                        work.tile([64, 512], f32, tag="recb", bufs=2, name="recb_b"),
                    ]

                    def stage2a(den2=den2, rec_row=rec_row):
                        # reciprocal on the 16-wide respread, regather to one
                        # partition row for the broadcast
                        rec2 = work.tile([64, 16], f32, tag="rec2", bufs=2)
                        nc.vector.reciprocal(out=rec2, in_=den2)
                        for hh in range(2):
                            nc.sync.dma_start(
                                out=rec_row[0:1, 512 * hh : 512 * hh + 512],
                                in_=rec2[:, 8 * hh : 8 * hh + 8],
                            )

                    def stage2b(rec_row=rec_row, recbs=recbs):
                        # GpSimd partition_broadcast fans 1/denom out across
                        # 64 partitions (SBUF APs can't have a 0 partition
                        # step, so a plain DVE op can't do this).
                        for hh in range(2):
                            nc.gpsimd.partition_broadcast(
                                out_ap=recbs[hh],
                                in_ap=rec_row[0:1, 512 * hh : 512 * hh + 512],
                            )

                    def stage3(recbs=recbs, sa=sa, sb2=sb2, pr=pr, c=c):
                        for hh, s_av in enumerate((sa, sb2)):
                            nc.vector.tensor_mul(
                                out=avt_sb[64 * hh : 64 * hh + 64, pr, ts(c, 512)],
                                in0=s_av[0:64, :],
                                in1=recbs[hh],
                            )

                    if c == 0:
                        # chunk-0 prs are only 4 steps (~5us) -- not enough
                        # ticks for the DMA chain latencies; space further out
                        norm_q.extend(
                            [None, stage2a, None, None, stage2b, None, None, stage3]
                        )
                    else:
                        norm_q.extend(
                            [None, stage2a, None, stage2b, None, stage3]
                        )

            # Filler plan (PE work interleaved into the attention steps, paced
            # so no chunk starves):
            #   prologue: QT/KT+V for chunk 0 (dense)
            #   chunk 0: QT/KT(1) + V tiles 4,5        (48 mms, 8 steps)
            #   chunk 1: V 6,7 + QT/KT(2) + V 8,9      (64 mms, 16 steps)
            #   chunk 2: V 10,11 + QT/KT(3) + V 12,13  (64 mms, 24 steps)
            #   chunk 3: V 14,15 + wo(0..2)            (64 mms, 32 steps)
            #   tail: wo(3)
            drain(qtkt_gen(0))
            drain(v_gen(range(0, 4)))
            plans = [
                (chain(qtkt_gen(1), v_gen([4, 5])), 5),
                (chain(v_gen([6, 7]), qtkt_gen(2), v_gen([8, 9])), 3),
                (chain(v_gen([10, 11]), qtkt_gen(3), v_gen([12, 13])), 2),
                (chain(v_gen([14, 15]), wo_gen(0), wo_gen(1), wo_gen(2)), 2),
            ]
            for c in range(NC_SQ):
                filler, quota = plans[c]
                attention_chunk(c, filler, quota)
                drain(filler)  # anything attention didn't absorb
            while norm_q:  # tail: flush the last pair's normalize stages
                st = norm_q.pop(0)
                if st is not None:
                    st()
            # dependency-free dummy matmuls keep the PE busy through the
            # final normalize chain so HAM doesn't re-throttle the clock to
            # 1.2GHz right before the last wo tile's matmuls
            pdum = psum.tile([128, 512], f32, tag="score", bufs=2)
            for k in range(36):
                nc.tensor.matmul(
                    pdum,
                    lhsT=wot_sb[:, 0, ts(0, 128)],
                    rhs=xt_sb[:, 0, ts(0, 512)],
                    start=(k == 0),
                    stop=(k == 35),
                )
            drain(wo_gen(NC_SQ - 1))

    nc.compile()
    return nc


def _get_nc():
    if "nc" not in _compiled:
        _compiled["nc"] = _build_nc()
    return _compiled["nc"]


def _swz(mat_t, kt):
    """[kt*128, m] -> [128, kt, m] contiguous (SBUF layout, d on partitions)."""
    import ml_dtypes

    m = mat_t.shape[1]
    return np.ascontiguousarray(
        mat_t.reshape(kt, 128, m).transpose(1, 0, 2)
    ).astype(ml_dtypes.bfloat16)


def make_in_maps(x, wq, bq, wk, bk, wv, bv, wo, bo):
    """Host-side sharding: core c handles batch c//TP, heads 4*(c%TP)..+4.
    Matrices are pre-swizzled to the exact SBUF layouts so device input DMAs
    are single fully-contiguous reads."""
    import ml_dtypes

    bf16 = ml_dtypes.bfloat16
    in_maps = []
    for c in range(NCORES):
        b = c // TP
        hs = (c % TP) * EC
        he = hs + EC
        xt = x[b].T  # [D, S]
        # [128, S//512 chunks, D//128 ktiles, 512]
        xt_swz = np.ascontiguousarray(
            xt.reshape(D // 128, 128, S // 512, 512).transpose(1, 2, 0, 3)
        ).astype(bf16)
        in_maps.append(
            {
                "xt": xt_swz,
                "wqt": _swz(np.ascontiguousarray(wq[hs:he, :].T), D // 128),
                "wkt": _swz(np.ascontiguousarray(wk[hs:he, :].T), D // 128),
                "wvt": _swz(np.ascontiguousarray(wv[hs:he, :].T), D // 128),
                "wot": _swz(np.ascontiguousarray(wo[:, hs:he].T), EC // 128),
                "bq": np.ascontiguousarray(bq[hs:he]),
                "bk": np.ascontiguousarray(bk[hs:he]),
            }
        )
    return in_maps


def combine_outputs(partials, wo, bv, bo):
    """Host-side unsharding: sum TP partials per batch + bias correction."""
    corr = (wo.astype(np.float32) @ bv.astype(np.float32)) + bo.astype(np.float32)
    out = np.zeros((B, S, D), dtype=np.float32)
    for b in range(B):
        acc = np.zeros((S, D), dtype=np.float32)
        for g in range(TP):
            acc += np.asarray(partials[b * TP + g], dtype=np.float32)
        out[b] = acc + corr[None, :]
    return out


def kernel(x, wq, bq, wk, bk, wv, bv, wo, bo):
    global LAST_EXEC_NS
    from concourse.bass_utils import run_bass_kernel_spmd

    x = np.asarray(x, dtype=np.float32)
    wq = np.asarray(wq, dtype=np.float32)
    bq = np.asarray(bq, dtype=np.float32)
    wk = np.asarray(wk, dtype=np.float32)
    bk = np.asarray(bk, dtype=np.float32)
    wv = np.asarray(wv, dtype=np.float32)
    bv = np.asarray(bv, dtype=np.float32)
    wo = np.asarray(wo, dtype=np.float32)
    bo = np.asarray(bo, dtype=np.float32)

    nc = _get_nc()
    in_maps = make_in_maps(x, wq, bq, wk, bk, wv, bv, wo, bo)
    res = run_bass_kernel_spmd(
        nc, in_maps, core_ids=list(range(NCORES)), trace=TRACE
    )
    LAST_EXEC_NS = res.exec_time_ns
    _compiled["last_res"] = res
    partials = [res.results[c]["out"] for c in range(NCORES)]
    return combine_outputs(partials, wo, bv, bo)


# revision 41
# speedup vs baseline: 1.1855x; 1.0039x over previous
"""Causal multi-head attention (b=2, s=2048, d=1024, h=16) on 8 TRN2 NeuronCores.

Sharding: DP=2 on batch x TP=4 on head groups (4 heads = 256 dims per core).
Host pre-transposes x and the weight slices so the device kernel is
transpose-free; the wo row-parallel partial sums + the bv/bo bias corrections
are applied on the host after gathering.

All matmul operands are bf16 (fp32 operands run in fp32_mode=HIGH, which
disables fast-weight-load and the background weight buffer -- every LDWEIGHTS
then serializes with the MM stream and row-group concurrency is lost; measured
335-423ns issue gaps at N=512 vs the 216ns bf16 floor).  PSUM accumulation
stays fp32; rel err vs the f32 oracle is ~3e-3 (budget 2e-2).

Device dataflow per core:
  xT [1024,2048] -> QT/KT [256,2048] (bias added on VectorE), V [2048,4x65]
  (65th column = ones, the stationary operand of the softmax denominator
  rows).  Per head pair and sq chunk: both heads' scoresT [sk,sq] land in one
  2-bank PSUM tile via concurrent row-group matmuls (head A in PE rows 0-63,
  B in 64-127), ONE merged exp per step on ScalarE (x1/8 folded into the
  activation scale) covering both heads, causal zeroing of the diag blocks on
  GpSimd post-exp, then the two AV+denominator matmuls accumulate per head.
  Softmax normalization runs as a deferred 3-stage pipeline (SBUF-local DMA
  respread -> 16-wide reciprocal -> GpSimd partition_broadcast -> muls), each
  stage emitted 1-2 attention steps after its inputs so no cross-engine wait
  ever parks at a FIFO queue head.

  The PE stream is issued in order; projection and wo matmuls are interleaved
  a few at a time between the scores and AV matmuls of every attention step
  to keep it dense through the ScalarE exp waits.
"""

import os

import numpy as np

D = 1024
S = 2048
B = 2
H = 16
DK = 64
TP = 4
DP = 2
EC = 256  # head dims per core
HPC = 4  # heads per core
NCORES = 8

TRACE = os.environ.get("KERNEL_TRACE", "0") == "1"
LAST_EXEC_NS = None

_compiled = {}


def _build_nc():
    import concourse.mybir as mybir
    from concourse import bacc, tile
    from concourse.bass import ts
    from itertools import chain

    f32 = mybir.dt.float32
    bf16 = mybir.dt.bfloat16
    AF = mybir.ActivationFunctionType

    nc = bacc.Bacc("TRN2", target_bir_lowering=False, debug=False)

    # Inputs are host-pre-swizzled to the exact SBUF layouts so every input
    # DMA is one fully-contiguous read (strided reads + many small DMAs made
    # the prologue DMA-latency-bound).
    xt_d = nc.dram_tensor(
        "xt", [128, S // 512, D // 128, 512], bf16, kind="ExternalInput"
    ).ap()
    wqt_d = nc.dram_tensor("wqt", [128, D // 128, EC], bf16, kind="ExternalInput").ap()
    wkt_d = nc.dram_tensor("wkt", [128, D // 128, EC], bf16, kind="ExternalInput").ap()
    wvt_d = nc.dram_tensor("wvt", [128, D // 128, EC], bf16, kind="ExternalInput").ap()
    wot_d = nc.dram_tensor("wot", [128, EC // 128, D], bf16, kind="ExternalInput").ap()
    bq_d = nc.dram_tensor("bq", [EC], f32, kind="ExternalInput").ap()
    bk_d = nc.dram_tensor("bk", [EC], f32, kind="ExternalInput").ap()
    out_d = nc.dram_tensor("out", [S, D], bf16, kind="ExternalOutput").ap()

    KT = D // 128  # 8 contraction tiles
    NC_SQ = S // 512  # 4 sq chunks

    with tile.TileContext(nc) as tc:
        with (
            tc.tile_pool(name="persist", bufs=1) as persist,
            tc.tile_pool(name="work", bufs=1) as work,
            tc.tile_pool(name="psum", bufs=1, space="PSUM") as psum,
            tc.tile_pool(name="dram", bufs=2, space="DRAM") as dram,
        ):
            # ---- persistent SBUF tensors ----
            xt_sb = persist.tile([128, KT, S], bf16)  # x^T, d on partitions
            wqt_sb = persist.tile([128, KT, EC], bf16)
            wkt_sb = persist.tile([128, KT, EC], bf16)
            wvt_sb = persist.tile([128, KT, EC], bf16)
            wot_sb = persist.tile([128, 2, D], bf16)
            bq_sb = persist.tile([128, 2], f32)
            bk_sb = persist.tile([128, 2], f32)
            qt_sb = persist.tile([128, 2, S], bf16)  # head pairs stacked
            kt_sb = persist.tile([128, 2, S], bf16)
            v_sb = persist.tile([128, S // 128, HPC * (DK + 1)], bf16)
            avt_sb = persist.tile([128, 2, S], bf16)

            # ---- input DMAs: one large contiguous transfer per tensor
            # (per-DMA fixed cost is ~0.6-2us and HWDGE drains FIFO, so many
            # small DMAs serialize), ordered so chunk-0 work starts ASAP ----
            nc.sync.dma_start(out=wqt_sb, in_=wqt_d)
            nc.sync.dma_start(out=xt_sb[:, :, ts(0, 512)], in_=xt_d[:, 0])
            nc.sync.dma_start(out=wkt_sb, in_=wkt_d)
            nc.sync.dma_start(out=wvt_sb, in_=wvt_d)
            nc.sync.dma_start(out=bq_sb, in_=bq_d.rearrange("(t p) -> p t", p=128))
            nc.sync.dma_start(out=bk_sb, in_=bk_d.rearrange("(t p) -> p t", p=128))
            for c in range(1, NC_SQ):
                nc.sync.dma_start(out=xt_sb[:, :, ts(c, 512)], in_=xt_d[:, c])
            nc.sync.dma_start(out=wot_sb, in_=wot_d)

            # ones column per head in V (stationary operand of the denom
            # matmul rows).
            v4 = v_sb.rearrange("p t (h e) -> p t h e", e=DK + 1)
            nc.vector.memset(v4[:, :, :, DK], 1.0)



            def qtkt_gen(c):
                """QT/KT projections for chunk c; yields once per matmul."""
                for dst_sb, w_sb, b_sb in (
                    (qt_sb, wqt_sb, bq_sb),
                    (kt_sb, wkt_sb, bk_sb),
                ):
                    for d2 in range(2):
                        ps = psum.tile([128, 512], f32, tag="proj", bufs=2)
                        for k in range(KT):
                            nc.tensor.matmul(
                                ps,
                                lhsT=w_sb[:, k, ts(d2, 128)],
                                rhs=xt_sb[:, k, ts(c, 512)],
                                start=(k == 0),
                                stop=(k == KT - 1),
                            )
                            if k == KT - 1:
                                nc.vector.tensor_scalar_add(
                                    out=dst_sb[:, d2, ts(c, 512)],
                                    in0=ps,
                                    scalar1=b_sb[:, d2 : d2 + 1],
                                )
                            yield

            def v_gen(tiles):
                """V projection for the given s-tiles; yields once per matmul."""
                for t in tiles:
                    ps = psum.tile([128, EC], f32, tag="proj", bufs=2)
                    for k in range(KT):
                        nc.tensor.matmul(
                            ps,
                            lhsT=xt_sb[:, k, ts(t, 128)],
                            rhs=wvt_sb[:, k, :],
                            start=(k == 0),
                            stop=(k == KT - 1),
                        )
                        if k == KT - 1:
                            nc.vector.tensor_copy(
                                out=v4[:, t, :, 0:DK],
                                in_=ps.rearrange("p (h e) -> p h e", e=DK),
                            )
                        yield

            def wo_gen(c):
                for t in range(4 * c, 4 * c + 4):
                    osb = work.tile([128, D], bf16, tag="osb", bufs=2)
                    for n in range(2):
                        po = psum.tile([128, 512], f32, tag="proj", bufs=2)
                        for p2 in range(2):
                            nc.tensor.matmul(
                                po,
                                lhsT=avt_sb[:, p2, ts(t, 128)],
                                rhs=wot_sb[:, p2, ts(n, 512)],
                                start=(p2 == 0),
                                stop=(p2 == 1),
                            )
                            if p2 == 1:
                                nc.vector.tensor_copy(
                                    out=osb[:, ts(n, 512)], in_=po
                                )
                                nc.sync.dma_start(
                                    out=out_d[ts(t, 128), ts(n, 512)],
                                    in_=osb[:, ts(n, 512)],
                                )
                            yield

            def drain(gen, n=None):
                took = 0
                for _ in gen:
                    took += 1
                    if n is not None and took >= n:
                        break
                return took

            # Deferred-normalize stages.  Every op in the chain (reciprocal,
            # broadcast, final muls) waits on a DMA or cross-engine result;
            # emitted inline they park at their engine's FIFO queue head and
            # block everything behind them (measured 6.7us DVE-head stalls
            # that starved the PE).  Instead each stage is emitted 1-2
            # attention steps AFTER its inputs were produced, so by the time
            # it reaches the queue head its inputs are long done.
            norm_q = []

            def attention_chunk(c, filler, quota):
                for pr in range(2):
                    pa = psum.tile([65, 512], f32, tag="av", bufs=2)
                    pb = psum.tile([65, 512], f32, tag="av", bufs=2)
                    n_sk = 4 * c + 4
                    for i in range(n_sk):
                        off = max(0, 128 * i - 512 * c)
                        w = 512 - off
                        sq_lo = 512 * c + off
                        # both heads' scoresT into one 2-bank psum tile:
                        # A via PE rows 0-63, B via rows 64-127 (concurrent)
                        pscore = psum.tile([128, 2, 512], f32, tag="score", bufs=2)
                        for hh, (p_lo, p_hi) in enumerate(((0, 64), (64, 128))):
                            nc.tensor.matmul(
                                pscore[:, hh, 0:w],
                                lhsT=kt_sb[p_lo:p_hi, pr, ts(i, 128)],
                                rhs=qt_sb[p_lo:p_hi, pr, sq_lo : sq_lo + w],
                                start=True,
                                stop=True,
                            )
                        et = work.tile([128, 2, 512], bf16, tag="exp", bufs=4)
                        nc.scalar.activation(
                            out=et[:, :, 0:w],
                            in_=pscore[:, :, 0:w],
                            func=AF.Exp,
                            scale=0.125,
                        )
                        if i >= 4 * c:
                            # causal: zero the lower triangle of the diag
                            # block post-exp on the otherwise-idle GpSimd
                            for hh in range(2):
                                dv = et[:, hh, 0:128]
                                nc.gpsimd.affine_select(
                                    out=dv,
                                    in_=dv,
                                    compare_op=mybir.AluOpType.is_ge,
                                    fill=0.0,
                                    base=0,
                                    pattern=[[1, 128]],
                                    channel_multiplier=-1,
                                )
                        # keep the in-order PE stream fed while the exp runs
                        q = quota + (2 if (i == 0 and not (c == 0 and pr == 0)) else 0)
                        drain(filler, q)
                        if norm_q:
                            st = norm_q.pop(0)
                            if st is not None:
                                st()
                        for hh, p_av in enumerate((pa, pb)):
                            h = 2 * pr + hh
                            nc.tensor.matmul(
                                p_av[:, off : off + w],
                                lhsT=v_sb[:, i, h * 65 : h * 65 + 65],
                                rhs=et[:, hh, 0:w],
                                start=(i == 0),
                                stop=(i == n_sk - 1),
                                skip_group_check=True,
                            )
                    # Stage 1, inline: copy the finished accumulators to SBUF
                    # (frees the two av PSUM banks after ~1.4us of DVE so the
                    # next pair's AV matmuls never wait on the chain), then
                    # SBUF->SBUF DMAs respread the denominator rows over 64
                    # partitions x 8 so the DVE iterative-divide reciprocal
                    # can run on a 16-wide free dim (290ns vs 3.3us at 512).
                    sa = work.tile([65, 512], f32, tag="avsb", bufs=4)
                    sb2 = work.tile([65, 512], f32, tag="avsb", bufs=4)
                    nc.vector.tensor_copy(out=sa, in_=pa)
                    nc.vector.tensor_copy(out=sb2, in_=pb)
                    den2 = work.tile([64, 16], f32, tag="den2", bufs=2)
                    nc.sync.dma_start(out=den2[:, 0:8], in_=sa[64:65, :])
                    nc.sync.dma_start(out=den2[:, 8:16], in_=sb2[64:65, :])
                    rec_row = work.tile([1, 1024], f32, tag="rec_row", bufs=2)
                    recbs = [
                        work.tile([64, 512], f32, tag="recb", bufs=2, name="recb_a"),
                        work.tile([64, 512], f32, tag="recb", bufs=2, name="recb_b"),
                    ]

                    def stage2a(den2=den2, rec_row=rec_row):
                        # reciprocal on the 16-wide respread, regather to one
                        # partition row for the broadcast
                        rec2 = work.tile([64, 16], f32, tag="rec2", bufs=2)
                        nc.vector.reciprocal(out=rec2, in_=den2)
                        for hh in range(2):
                            nc.sync.dma_start(
                                out=rec_row[0:1, 512 * hh : 512 * hh + 512],
                                in_=rec2[:, 8 * hh : 8 * hh + 8],
                            )

                    def stage2b(rec_row=rec_row, recbs=recbs):
                        # GpSimd partition_broadcast fans 1/denom out across
                        # 64 partitions (SBUF APs can't have a 0 partition
                        # step, so a plain DVE op can't do this).
                        for hh in range(2):
                            nc.gpsimd.partition_broadcast(
                                out_ap=recbs[hh],
                                in_ap=rec_row[0:1, 512 * hh : 512 * hh + 512],
                            )

                    def stage3(recbs=recbs, sa=sa, sb2=sb2, pr=pr, c=c):
                        for hh, s_av in enumerate((sa, sb2)):
                            nc.vector.tensor_mul(
                                out=avt_sb[64 * hh : 64 * hh + 64, pr, ts(c, 512)],
                                in0=s_av[0:64, :],
                                in1=recbs[hh],
                            )

                    if c == 0:
                        # chunk-0 prs are only 4 steps: stage2a popped inside
                        # the next pr would sit in the DVE stream BEFORE that
                        # pr's accumulator copies, head-waiting on its DMA
                        # respread and delaying the very copies the following
                        # chunk's AV matmuls need.  Defer the whole chain past
                        # the next pr's end (chunk-0 avt has until chunk 3).
                        norm_q.extend(
                            [None] * 4
                            + [stage2a, None, stage2b, None, None, stage3]
                        )
                    else:
                        norm_q.extend(
                            [None, stage2a, None, stage2b, None, stage3]
                        )

            # Filler plan (PE work interleaved into the attention steps, paced
            # so no chunk starves):
            #   prologue: QT/KT+V for chunk 0 (dense)
            #   chunk 0: QT/KT(1) + V tiles 4,5        (48 mms, 8 steps)
            #   chunk 1: V 6,7 + QT/KT(2) + V 8,9      (64 mms, 16 steps)
            #   chunk 2: V 10,11 + QT/KT(3) + V 12,13  (64 mms, 24 steps)
            #   chunk 3: V 14,15 + wo(0..2)            (64 mms, 32 steps)
            #   tail: wo(3)
            drain(qtkt_gen(0))
            drain(v_gen(range(0, 4)))
            plans = [
                (chain(qtkt_gen(1), v_gen([4, 5])), 5),
                (chain(v_gen([6, 7]), qtkt_gen(2), v_gen([8, 9])), 3),
                (chain(v_gen([10, 11]), qtkt_gen(3), v_gen([12, 13])), 2),
                (chain(v_gen([14, 15]), wo_gen(0), wo_gen(1), wo_gen(2)), 2),
            ]
            for c in range(NC_SQ):
                filler, quota = plans[c]
                attention_chunk(c, filler, quota)
                drain(filler)  # anything attention didn't absorb
            while norm_q:  # tail: flush the last pair's normalize stages
                st = norm_q.pop(0)
                if st is not None:
                    st()
            # dependency-free dummy matmuls keep the PE busy through the
            # final normalize chain so HAM doesn't re-throttle the clock to
            # 1.2GHz right before the last wo tile's matmuls
            pdum = psum.tile([128, 512], f32, tag="score", bufs=2)
            for k in range(36):
                nc.tensor.matmul(
                    pdum,
                    lhsT=wot_sb[:, 0, ts(0, 128)],
                    rhs=xt_sb[:, 0, ts(0, 512)],
                    start=(k == 0),
                    stop=(k == 35),
                )
            drain(wo_gen(NC_SQ - 1))

    nc.compile()
    return nc


def _get_nc():
    if "nc" not in _compiled:
        _compiled["nc"] = _build_nc()
    return _compiled["nc"]


def _swz(mat_t, kt):
    """[kt*128, m] -> [128, kt, m] contiguous (SBUF layout, d on partitions)."""
    import ml_dtypes

    m = mat_t.shape[1]
    return np.ascontiguousarray(
        mat_t.reshape(kt, 128, m).transpose(1, 0, 2)
    ).astype(ml_dtypes.bfloat16)


def make_in_maps(x, wq, bq, wk, bk, wv, bv, wo, bo):
    """Host-side sharding: core c handles batch c//TP, heads 4*(c%TP)..+4.
    Matrices are pre-swizzled to the exact SBUF layouts so device input DMAs
    are single fully-contiguous reads."""
    import ml_dtypes

    bf16 = ml_dtypes.bfloat16
    in_maps = []
    for c in range(NCORES):
        b = c // TP
        hs = (c % TP) * EC
        he = hs + EC
        xt = x[b].T  # [D, S]
        # [128, S//512 chunks, D//128 ktiles, 512]
        xt_swz = np.ascontiguousarray(
            xt.reshape(D // 128, 128, S // 512, 512).transpose(1, 2, 0, 3)
        ).astype(bf16)
        in_maps.append(
            {
                "xt": xt_swz,
                "wqt": _swz(np.ascontiguousarray(wq[hs:he, :].T), D // 128),
                "wkt": _swz(np.ascontiguousarray(wk[hs:he, :].T), D // 128),
                "wvt": _swz(np.ascontiguousarray(wv[hs:he, :].T), D // 128),
                "wot": _swz(np.ascontiguousarray(wo[:, hs:he].T), EC // 128),
                "bq": np.ascontiguousarray(bq[hs:he]),
                "bk": np.ascontiguousarray(bk[hs:he]),
            }
        )
    return in_maps


def combine_outputs(partials, wo, bv, bo):
    """Host-side unsharding: sum TP partials per batch + bias correction."""
    corr = (wo.astype(np.float32) @ bv.astype(np.float32)) + bo.astype(np.float32)
    out = np.zeros((B, S, D), dtype=np.float32)
    for b in range(B):
        acc = np.zeros((S, D), dtype=np.float32)
        for g in range(TP):
            acc += np.asarray(partials[b * TP + g], dtype=np.float32)
        out[b] = acc + corr[None, :]
    return out


def kernel(x, wq, bq, wk, bk, wv, bv, wo, bo):
    global LAST_EXEC_NS
    from concourse.bass_utils import run_bass_kernel_spmd

    x = np.asarray(x, dtype=np.float32)
    wq = np.asarray(wq, dtype=np.float32)
    bq = np.asarray(bq, dtype=np.float32)
    wk = np.asarray(wk, dtype=np.float32)
    bk = np.asarray(bk, dtype=np.float32)
    wv = np.asarray(wv, dtype=np.float32)
    bv = np.asarray(bv, dtype=np.float32)
    wo = np.asarray(wo, dtype=np.float32)
    bo = np.asarray(bo, dtype=np.float32)

    nc = _get_nc()
    in_maps = make_in_maps(x, wq, bq, wk, bk, wv, bv, wo, bo)
    res = run_bass_kernel_spmd(
        nc, in_maps, core_ids=list(range(NCORES)), trace=TRACE
    )
    LAST_EXEC_NS = res.exec_time_ns
    _compiled["last_res"] = res
    partials = [res.results[c]["out"] for c in range(NCORES)]
    return combine_outputs(partials, wo, bv, bo)
